# revision 18
# baseline (speedup 1.0000x reference)
"""Trainium2 kernel for nn_Entropy_55525337203040 (retrieval kNN entropy).

Strategy (8 NeuronCores, SPMD):
  - Shard gallery (20000 rows) along Ng: 2500 rows per core.
  - Per core: approximate selection scores
        sel[q, g] = q.g - (||g||^2/2 - 1024)
    computed entirely on the tensor engine: fp8e4 DoubleRow matmuls (2
    k-tiles of 128 contracted per instruction, fp32 PSUM accumulate) give
    2x the fp16 FLOP rate; the per-gallery-column correction is folded in
    as one extra K=2 plain-fp8 matmul per PSUM tile (hi/lo fp8 split of
    -(g2/2 - 1024), query side = 1.0, error < 0.05).
  - Per PSUM tile [128 queries, <=512 gallery cols]: DVE max8 + max_index
    extract the top-8 (value, index) pairs of the chunk.  No other vector
    work: PSUM is read exactly twice.
  - Selection is approximate (fp8 dot error sigma ~2.3); exact values are
    recovered on the host: decode chunk-local indices to gallery rows,
    keep the best TOPR=40 of the 8x5x8=320 candidates per query by
    approximate score, recompute their exact logits 2 q.g - ||g||^2 from
    the original fp32 inputs, then exact top-k + log-softmax entropy in
    fp64.  Validated: rel err ~1e-5, zero weight-bearing candidates
    missed across seeds (tolerance 2e-2).
"""

import numpy as np
import ml_dtypes

NQ, NG, D, K = 256, 20000, 2048, 32
M = 8                 # cores
SH = NG // M          # 2500 gallery rows per core
P = 128
KT = D // P           # 16 contraction tiles of 128
KP = KT // 2          # 8 DoubleRow k-pairs of 256
NT = 512              # gallery columns per psum tile (PSUM bank = 512 fp32)
CK = 8                # candidates kept per chunk (max8)
NCH = -(-SH // NT)    # 5 chunks of 512
SHP = NCH * NT        # 2560: shard padded to uniform chunks; pad columns get
                      # sel = -480 (zero features, combo rows -240) so they
                      # can never enter a chunk's top-8
TOPR = 40             # candidates rescued exactly per query on host

_CACHE = {}


def build_program(reps=1, n_warmup=8, pair_chunks=False, split_c0=8,
                  gal_bufs=4, psum_bufs=6, kp_sub=None, no_dve=False,
                  no_mm=False, all_chunks=False, hw_loop=0):
    import concourse.bass as bass
    import concourse.tile as tile
    from concourse import bacc, mybir

    f8 = mybir.dt.float8e4
    f32 = mybir.dt.float32
    u16 = mybir.dt.uint16
    DR = mybir.MatmulPerfMode.DoubleRow
    kp_use = KP if kp_sub is None else kp_sub

    nc = bacc.Bacc(
        "TRN2",
        target_bir_lowering=False,
        debug=False,
        num_devices=M,
    )

    # gt is chunk-major: each [P, KT, NT] chunk is a contiguous 1 MB block
    # (8 KB per partition line) so the gallery DMA is byte-bound, not
    # descriptor-line-bound.
    qt = nc.dram_tensor("qt", [P, KT, NQ], f8, kind="ExternalInput").ap()
    gt = nc.dram_tensor("gt", [NCH, P, KT, NT], f8, kind="ExternalInput").ap()
    cbn = nc.dram_tensor("cbn", [2, SHP], f8, kind="ExternalInput").ap()
    candv = nc.dram_tensor("candv", [2, P, NCH * CK], f32, kind="ExternalOutput").ap()
    candi = nc.dram_tensor("candi", [2, P, NCH * CK], u16, kind="ExternalOutput").ap()

    with tile.TileContext(nc) as tc:
        with (
            tc.tile_pool(name="const", bufs=1) as const_pool,
            tc.tile_pool(name="gal", bufs=gal_bufs) as gal_pool,
            tc.tile_pool(name="psum", bufs=psum_bufs, space="PSUM") as psum_pool,
            tc.tile_pool(name="cand", bufs=2) as cand_pool,
        ):
            # PE warmup: matmuls on a zeroed tile, independent of any DMA,
            # keep the PE HAM busy during the pipeline-fill DMAs so the real
            # matmuls run at 2.4 GHz from the start.
            if n_warmup:
                wu_in = const_pool.tile([P, NT], f8, tag="wu_in")
                nc.vector.memset(wu_in[:], 0.0)
                wu_ps = psum_pool.tile([P, NT], f32, tag="wu_ps", bufs=1)
                for w in range(n_warmup):
                    nc.tensor.matmul(
                        wu_ps[:], wu_in[:, :P], wu_in[:], start=True, stop=True
                    )

            # queries + negated combo rows on the gpsimd SWDGE queues so they
            # run in parallel with the gallery chunks on the sync HWDGE ring
            qt_sb = const_pool.tile([P, KT, NQ], f8, tag="qt_sb")
            nc.gpsimd.dma_start(out=qt_sb[:], in_=qt[:])
            cbn_sb = const_pool.tile([2, SHP], f8, tag="cbn_sb")
            nc.gpsimd.dma_start(out=cbn_sb[:], in_=cbn[:])
            ones8 = const_pool.tile([2, P], f8, tag="ones8")
            nc.vector.memset(ones8[:], 1.0)

            def one_rep(r):
                candv_sb = [
                    cand_pool.tile([P, NCH * CK], f32, tag=f"cv{m}",
                                   name=f"cv{r}_{m}")
                    for m in range(2)
                ]
                candi_sb = [
                    cand_pool.tile([P, NCH * CK], u16, tag=f"ci{m}",
                                   name=f"ci{r}_{m}")
                    for m in range(2)
                ]

                def load_chunk(j, split):
                    g_sb = gal_pool.tile([P, KT, NT], f8, tag="g_sb",
                                         name=f"g_sb{r}_{j}")
                    if split:
                        bounds = list(range(0, KT, max(1, KT // split)))
                        bounds.append(KT)
                        for a, b in zip(bounds[:-1], bounds[1:]):
                            nc.sync.dma_start(
                                out=g_sb[:, a:b, :], in_=gt[j, :, a:b, :]
                            )
                    else:
                        nc.sync.dma_start(out=g_sb[:], in_=gt[j])
                    return g_sb

                def compute_group(grp, loaded):
                    """One group of chunks, both query halves.  The combo
                    (K=2) matmul shares one weight load across all psum tiles
                    of the group; each DoubleRow weight load serves all
                    chunks in the group."""
                    pss = {
                        (m, j): psum_pool.tile([P, NT], f32, tag="ps",
                                               name=f"ps{r}_{j}_{m}")
                        for j in grp for m in range(2)
                    }
                    if not no_mm:
                        first = True
                        for j in grp:
                            for m in range(2):
                                mm = nc.tensor.matmul(
                                    pss[m, j][:], ones8[:],
                                    cbn_sb[:, bass.ts(j, NT)],
                                    start=True, stop=False,
                                )
                                if not first:
                                    mm.ldweights = False
                                first = False
                        for k in range(kp_use):
                            for m in range(2):
                                fresh = True
                                for j in grp:
                                    mm = nc.tensor.matmul(
                                        pss[m, j][:],
                                        qt_sb[:, 2 * k:2 * k + 2, bass.ts(m, P)],
                                        loaded[j][:, 2 * k:2 * k + 2, :],
                                        start=False,
                                        stop=(k == kp_use - 1),
                                        perf_mode=DR,
                                    )
                                    if not fresh:
                                        mm.ldweights = False
                                    fresh = False
                    for j in grp:
                        for m in range(2):
                            nc.vector.max(candv_sb[m][:, bass.ts(j, CK)],
                                          pss[m, j][:])
                            if not no_dve:
                                nc.vector.max_index(
                                    candi_sb[m][:, bass.ts(j, CK)],
                                    candv_sb[m][:, bass.ts(j, CK)],
                                    pss[m, j][:],
                                )

                if all_chunks:
                    groups = [list(range(NCH))]
                elif pair_chunks:
                    groups = [[0, 1], [2, 3], [4]]
                else:
                    groups = [[j] for j in range(NCH)]
                loaded = {}
                for grp in groups:
                    for j in grp:
                        loaded[j] = load_chunk(j, split_c0 if (j == 0 and r == 0) else 0)
                    compute_group(grp, loaded)

                for m in range(2):
                    nc.sync.dma_start(out=candv[m], in_=candv_sb[m][:])
                    if not no_dve:
                        nc.sync.dma_start(out=candi[m], in_=candi_sb[m][:])

            if hw_loop:
                with tc.For_i(0, hw_loop) as _i:
                    for r in range(reps):
                        one_rep(r)
            else:
                for r in range(reps):
                    one_rep(r)

    nc.compile()
    return nc


def prep_inputs(feat, gallery):
    """Host-side prep: cast fp8e4, transpose to [partition, ktile, col]
    layout, compute negated combo hi/lo rows, shard gallery across cores."""
    feat = np.asarray(feat, np.float32)
    gallery = np.asarray(gallery, np.float32)

    q8 = feat.astype(ml_dtypes.float8_e4m3)                     # [NQ, D]
    qt_host = np.ascontiguousarray(
        q8.reshape(NQ, KT, P).transpose(2, 1, 0)                # [P, KT, NQ]
    )

    g8 = gallery.astype(ml_dtypes.float8_e4m3)                  # [NG, D]
    g2f = (gallery.astype(np.float64) ** 2).sum(1).astype(np.float32)
    combo = -(g2f / 2.0 - 1024.0).astype(np.float32)            # negated
    hi = combo.astype(ml_dtypes.float8_e4m3)
    lo = (combo - hi.astype(np.float32)).astype(ml_dtypes.float8_e4m3)
    cbn_host = np.stack([hi, lo])                               # [2, NG] f8

    in_maps = []
    pad = np.zeros((SHP - SH, D), ml_dtypes.float8_e4m3)
    cbn_pad = np.full((2, SHP - SH), -240.0, ml_dtypes.float8_e4m3)
    for c in range(M):
        shard = np.concatenate([g8[c * SH:(c + 1) * SH], pad])  # [SHP, D]
        # chunk-major: [NCH, P, KT, NT], each chunk contiguous
        gt_c = np.ascontiguousarray(
            shard.reshape(NCH, NT, KT, P).transpose(0, 3, 2, 1)
        )
        cbn_c = np.ascontiguousarray(np.concatenate(
            [cbn_host[:, c * SH:(c + 1) * SH], cbn_pad], axis=1
        ))
        in_maps.append({"qt": qt_host, "gt": gt_c, "cbn": cbn_c})
    return in_maps


def merge_outputs(cvs, cis, k, feat, gallery):
    """cvs/cis: per-core [2, P, NCH*CK] approx values (fp32) and chunk-local
    indices (uint16) -> exact host rescue -> mean entropy (fp32 scalar)."""
    feat = np.asarray(feat, np.float32)
    gallery = np.asarray(gallery, np.float32)
    g2f = (gallery.astype(np.float64) ** 2).sum(1).astype(np.float32)

    off = np.repeat(np.arange(NCH, dtype=np.int64) * NT, CK)    # [NCH*CK]
    vals, idxs = [], []
    for c in range(M):
        v = np.asarray(cvs[c], np.float32).reshape(NQ, NCH * CK)
        i = np.asarray(cis[c], np.int64).reshape(NQ, NCH * CK)
        gi = i + off[None, :]
        # pad columns (>= SH) carry sel=-480 and are never selected; clamp
        # defensively so a stray index cannot go out of bounds
        bad = gi >= SH
        if bad.any():
            v = v.copy()
            v[bad] = -np.inf
            gi = np.minimum(gi, SH - 1)
        idxs.append(gi + c * SH)
        vals.append(v)
    allv = np.concatenate(vals, axis=1)                         # [NQ, M*NCH*CK]
    alli = np.concatenate(idxs, axis=1)

    topr = min(TOPR, allv.shape[1])
    order = np.argpartition(-allv, topr - 1, axis=1)[:, :topr]
    ridx = np.take_along_axis(alli, order, axis=1)              # [NQ, topr]

    # exact logits 2 q.g - g2 for the rescued candidates
    gsel = gallery[ridx]                                        # [NQ, topr, D]
    dots = np.matmul(gsel, feat[:, :, None])[:, :, 0]           # [NQ, topr] f32
    exact = 2.0 * dots.astype(np.float64) - g2f[ridx].astype(np.float64)

    kk = min(int(k), topr)
    part = np.argpartition(-exact, kk - 1, axis=1)[:, :kk]
    top = np.take_along_axis(exact, part, axis=1)
    sh = top - top.max(1, keepdims=True)
    logp = sh - np.log(np.exp(sh).sum(1, keepdims=True))
    p = np.exp(logp)
    ent = -(p * logp).sum(1)
    return np.float32(ent.mean())


def kernel(feat, gallery_features, k):
    from concourse.bass_utils import run_bass_kernel_spmd

    if "nc" not in _CACHE:
        _CACHE["nc"] = build_program()
    nc = _CACHE["nc"]

    in_maps = prep_inputs(feat, gallery_features)
    res = run_bass_kernel_spmd(nc, in_maps, list(range(M)))
    cvs = [res.results[c]["candv"] for c in range(M)]
    cis = [res.results[c]["candi"] for c in range(M)]
    return merge_outputs(cvs, cis, k, feat, gallery_features)


# revision 32
# speedup vs baseline: 6.1364x; 6.1364x over previous
"""Trainium2 kernel for nn_Entropy_55525337203040 (retrieval kNN entropy).

Strategy (8 NeuronCores, SPMD):
  - Shard gallery (20000 rows) along Ng: 2500 rows per core.
  - Per core: approximate selection scores
        sel[q, g] = q.g - (||g||^2/2 - 1024)
    computed entirely on the tensor engine: fp8e4 DoubleRow matmuls (2
    k-tiles of 128 contracted per instruction, fp32 PSUM accumulate) give
    2x the fp16 FLOP rate; the per-gallery-column correction is folded in
    as one extra K=2 plain-fp8 matmul per PSUM tile (hi/lo fp8 split of
    -(g2/2 - 1024), query side = 1.0, error < 0.05).
  - Per PSUM tile [128 queries, <=512 gallery cols]: DVE max8 + max_index
    extract the top-8 (value, index) pairs of the chunk.  No other vector
    work: PSUM is read exactly twice.
  - Selection is approximate (fp8 dot error sigma ~2.3); exact values are
    recovered on the host: decode chunk-local indices to gallery rows,
    keep the best TOPR=40 of the 8x5x8=320 candidates per query by
    approximate score, recompute their exact logits 2 q.g - ||g||^2 from
    the original fp32 inputs, then exact top-k + log-softmax entropy in
    fp64.  Validated: rel err ~1e-5, zero weight-bearing candidates
    missed across seeds (tolerance 2e-2).
"""

import numpy as np
import ml_dtypes

NQ, NG, D, K = 256, 20000, 2048, 32
M = 8                 # cores
SH = NG // M          # 2500 gallery rows per core
P = 128
KT = D // P           # 16 contraction tiles of 128
KP = KT // 2          # 8 DoubleRow k-pairs of 256
NT = 512              # gallery columns per psum tile (PSUM bank = 512 fp32)
CK = 8                # candidates kept per chunk (max8)
NCH = -(-SH // NT)    # 5 chunks of 512
SHP = NCH * NT        # 2560: shard padded to uniform chunks; pad columns get
                      # sel = -480 (zero features, combo rows -240) so they
                      # can never enter a chunk's top-8
TOPR = 40             # candidates rescued exactly per query on host

_CACHE = {}


def build_program(reps=1, n_warmup=8, pair_chunks=False, split_c0=8,
                  gal_bufs=4, psum_bufs=6, kp_sub=None, no_dve=False,
                  no_mm=False, all_chunks=False, hw_loop=0, dual_ring=True,
                  resident_gal=False, act_copy=False):
    import concourse.bass as bass
    import concourse.tile as tile
    from concourse import bacc, mybir

    f8 = mybir.dt.float8e4
    f32 = mybir.dt.float32
    bf16 = mybir.dt.bfloat16
    u16 = mybir.dt.uint16
    DR = mybir.MatmulPerfMode.DoubleRow
    kp_use = KP if kp_sub is None else kp_sub
    cand_dt = bf16 if act_copy else f32

    nc = bacc.Bacc(
        "TRN2",
        target_bir_lowering=False,
        debug=False,
        num_devices=M,
    )

    # gt is chunk-major: each [P, KT, NT] chunk is a contiguous 1 MB block
    # (8 KB per partition line) so the gallery DMA is byte-bound, not
    # descriptor-line-bound.
    qt = nc.dram_tensor("qt", [P, KT, NQ], f8, kind="ExternalInput").ap()
    gt = nc.dram_tensor("gt", [NCH, P, KT, NT], f8, kind="ExternalInput").ap()
    cbn = nc.dram_tensor("cbn", [2, SHP], f8, kind="ExternalInput").ap()
    candv = nc.dram_tensor("candv", [2, P, NCH * CK], cand_dt,
                           kind="ExternalOutput").ap()
    candi = nc.dram_tensor("candi", [2, P, NCH * CK], u16, kind="ExternalOutput").ap()

    with tile.TileContext(nc) as tc:
        with (
            tc.tile_pool(name="const", bufs=1) as const_pool,
            tc.tile_pool(name="gal", bufs=gal_bufs) as gal_pool,
            tc.tile_pool(name="psum", bufs=psum_bufs, space="PSUM") as psum_pool,
            tc.tile_pool(name="cand", bufs=2) as cand_pool,
            tc.tile_pool(name="sel", bufs=4) as sel_pool,
        ):
            # PE warmup: matmuls on a zeroed tile, independent of any DMA,
            # keep the PE HAM busy during the pipeline-fill DMAs so the real
            # matmuls run at 2.4 GHz from the start.
            if n_warmup:
                wu_in = const_pool.tile([P, NT], f8, tag="wu_in")
                nc.vector.memset(wu_in[:], 0.0)
                wu_ps = psum_pool.tile([P, NT], f32, tag="wu_ps", bufs=1)
                for w in range(n_warmup):
                    nc.tensor.matmul(
                        wu_ps[:], wu_in[:, :P], wu_in[:], start=True, stop=True
                    )

            # queries + negated combo rows on the gpsimd SWDGE queues so they
            # run in parallel with the gallery chunks on the sync HWDGE ring
            qt_sb = const_pool.tile([P, KT, NQ], f8, tag="qt_sb")
            nc.gpsimd.dma_start(out=qt_sb[:], in_=qt[:])
            cbn_sb = const_pool.tile([2, SHP], f8, tag="cbn_sb")
            nc.gpsimd.dma_start(out=cbn_sb[:], in_=cbn[:])
            ones8 = const_pool.tile([2, P], f8, tag="ones8")
            nc.vector.memset(ones8[:], 1.0)

            res_gal = {}
            if resident_gal:
                for j in range(NCH):
                    g_sb = const_pool.tile([P, KT, NT], f8, tag=f"rg{j}")
                    nc.sync.dma_start(out=g_sb[:], in_=gt[j])
                    res_gal[j] = g_sb

            def one_rep(r):
                candv_sb = [
                    cand_pool.tile([P, NCH * CK], cand_dt, tag=f"cv{m}",
                                   name=f"cv{r}_{m}")
                    for m in range(2)
                ]
                candi_sb = [
                    cand_pool.tile([P, NCH * CK], u16, tag=f"ci{m}",
                                   name=f"ci{r}_{m}")
                    for m in range(2)
                ]

                def load_chunk(j, split):
                    g_sb = gal_pool.tile([P, KT, NT], f8, tag="g_sb",
                                         name=f"g_sb{r}_{j}")
                    eng = nc.scalar if (dual_ring and j % 2) else nc.sync
                    if split:
                        bounds = list(range(0, KT, max(1, KT // split)))
                        bounds.append(KT)
                        for a, b in zip(bounds[:-1], bounds[1:]):
                            eng.dma_start(
                                out=g_sb[:, a:b, :], in_=gt[j, :, a:b, :]
                            )
                    else:
                        eng.dma_start(out=g_sb[:], in_=gt[j])
                    return g_sb

                def compute_group(grp, loaded):
                    """One group of chunks, both query halves.  The combo
                    (K=2) matmul shares one weight load across all psum tiles
                    of the group; each DoubleRow weight load serves all
                    chunks in the group."""
                    if no_mm:
                        return
                    pss = {
                        (m, j): psum_pool.tile([P, NT], f32, tag="ps",
                                               name=f"ps{r}_{j}_{m}")
                        for j in grp for m in range(2)
                    }
                    first = True
                    for j in grp:
                        for m in range(2):
                            mm = nc.tensor.matmul(
                                pss[m, j][:], ones8[:],
                                cbn_sb[:, bass.ts(j, NT)],
                                start=True, stop=False,
                            )
                            if not first:
                                mm.ldweights = False
                            first = False
                    for k in range(kp_use):
                        for m in range(2):
                            fresh = True
                            for j in grp:
                                mm = nc.tensor.matmul(
                                    pss[m, j][:],
                                    qt_sb[:, 2 * k:2 * k + 2, bass.ts(m, P)],
                                    loaded[j][:, 2 * k:2 * k + 2, :],
                                    start=False,
                                    stop=(k == kp_use - 1),
                                    perf_mode=DR,
                                )
                                if not fresh:
                                    mm.ldweights = False
                                fresh = False
                    for j in grp:
                        for m in range(2):
                            if no_dve == "all":
                                # timing probe: drain psum with a minimal read
                                nc.vector.max(candv_sb[m][:, bass.ts(j, CK)],
                                              pss[m, j][:, :CK])
                                continue
                            if act_copy:
                                # ACT copies PSUM->SBUF bf16 (otherwise-idle
                                # engine); DVE then streams 2-byte SBUF data
                                # at its fast path instead of fp32 PSUM.
                                selb = sel_pool.tile(
                                    [P, NT], bf16, tag="selb",
                                    name=f"selb{r}_{j}_{m}")
                                nc.scalar.activation(
                                    selb[:], pss[m, j][:],
                                    mybir.ActivationFunctionType.Copy,
                                )
                                src = selb
                            else:
                                src = pss[m, j]
                            nc.vector.max(candv_sb[m][:, bass.ts(j, CK)],
                                          src[:])
                            if not no_dve:
                                nc.vector.max_index(
                                    candi_sb[m][:, bass.ts(j, CK)],
                                    candv_sb[m][:, bass.ts(j, CK)],
                                    src[:],
                                )

                if no_mm:
                    for m in range(2):
                        nc.vector.memset(candv_sb[m][:], 0.0)
                if all_chunks:
                    groups = [list(range(NCH))]
                elif pair_chunks:
                    groups = [[0, 1], [2, 3], [4]]
                else:
                    groups = [[j] for j in range(NCH)]
                loaded = {}
                for grp in groups:
                    for j in grp:
                        if resident_gal:
                            loaded[j] = res_gal[j]
                        else:
                            loaded[j] = load_chunk(
                                j, split_c0 if (j == 0 and r == 0) else 0)
                    compute_group(grp, loaded)

                for m in range(2):
                    nc.sync.dma_start(out=candv[m], in_=candv_sb[m][:])
                    if not (no_dve or no_mm):
                        nc.sync.dma_start(out=candi[m], in_=candi_sb[m][:])

            if hw_loop:
                with tc.For_i(0, hw_loop) as _i:
                    for r in range(reps):
                        one_rep(r)
            else:
                for r in range(reps):
                    one_rep(r)

    nc.compile()
    return nc


def prep_inputs(feat, gallery):
    """Host-side prep: cast fp8e4, transpose to [partition, ktile, col]
    layout, compute negated combo hi/lo rows, shard gallery across cores."""
    feat = np.asarray(feat, np.float32)
    gallery = np.asarray(gallery, np.float32)

    q8 = feat.astype(ml_dtypes.float8_e4m3)                     # [NQ, D]
    qt_host = np.ascontiguousarray(
        q8.reshape(NQ, KT, P).transpose(2, 1, 0)                # [P, KT, NQ]
    )

    g8 = gallery.astype(ml_dtypes.float8_e4m3)                  # [NG, D]
    g2f = (gallery.astype(np.float64) ** 2).sum(1).astype(np.float32)
    combo = -(g2f / 2.0 - 1024.0).astype(np.float32)            # negated
    hi = combo.astype(ml_dtypes.float8_e4m3)
    lo = (combo - hi.astype(np.float32)).astype(ml_dtypes.float8_e4m3)
    cbn_host = np.stack([hi, lo])                               # [2, NG] f8

    in_maps = []
    pad = np.zeros((SHP - SH, D), ml_dtypes.float8_e4m3)
    cbn_pad = np.full((2, SHP - SH), -240.0, ml_dtypes.float8_e4m3)
    for c in range(M):
        shard = np.concatenate([g8[c * SH:(c + 1) * SH], pad])  # [SHP, D]
        # chunk-major: [NCH, P, KT, NT], each chunk contiguous
        gt_c = np.ascontiguousarray(
            shard.reshape(NCH, NT, KT, P).transpose(0, 3, 2, 1)
        )
        cbn_c = np.ascontiguousarray(np.concatenate(
            [cbn_host[:, c * SH:(c + 1) * SH], cbn_pad], axis=1
        ))
        in_maps.append({"qt": qt_host, "gt": gt_c, "cbn": cbn_c})
    return in_maps


def merge_outputs(cvs, cis, k, feat, gallery):
    """cvs/cis: per-core [2, P, NCH*CK] approx values (fp32) and chunk-local
    indices (uint16) -> exact host rescue -> mean entropy (fp32 scalar)."""
    feat = np.asarray(feat, np.float32)
    gallery = np.asarray(gallery, np.float32)
    g2f = (gallery.astype(np.float64) ** 2).sum(1).astype(np.float32)

    off = np.repeat(np.arange(NCH, dtype=np.int64) * NT, CK)    # [NCH*CK]
    vals, idxs = [], []
    for c in range(M):
        v = np.asarray(cvs[c], np.float32).reshape(NQ, NCH * CK)
        i = np.asarray(cis[c], np.int64).reshape(NQ, NCH * CK)
        gi = i + off[None, :]
        # pad columns (>= SH) carry sel=-480 and are never selected; clamp
        # defensively so a stray index cannot go out of bounds
        bad = gi >= SH
        if bad.any():
            v = v.copy()
            v[bad] = -np.inf
            gi = np.minimum(gi, SH - 1)
        idxs.append(gi + c * SH)
        vals.append(v)
    allv = np.concatenate(vals, axis=1)                         # [NQ, M*NCH*CK]
    alli = np.concatenate(idxs, axis=1)

    topr = min(TOPR, allv.shape[1])
    order = np.argpartition(-allv, topr - 1, axis=1)[:, :topr]
    ridx = np.take_along_axis(alli, order, axis=1)              # [NQ, topr]

    # exact logits 2 q.g - g2 for the rescued candidates
    gsel = gallery[ridx]                                        # [NQ, topr, D]
    dots = np.matmul(gsel, feat[:, :, None])[:, :, 0]           # [NQ, topr] f32
    exact = 2.0 * dots.astype(np.float64) - g2f[ridx].astype(np.float64)

    kk = min(int(k), topr)
    part = np.argpartition(-exact, kk - 1, axis=1)[:, :kk]
    top = np.take_along_axis(exact, part, axis=1)
    sh = top - top.max(1, keepdims=True)
    logp = sh - np.log(np.exp(sh).sum(1, keepdims=True))
    p = np.exp(logp)
    ent = -(p * logp).sum(1)
    return np.float32(ent.mean())


def kernel(feat, gallery_features, k):
    from concourse.bass_utils import run_bass_kernel_spmd

    if "nc" not in _CACHE:
        _CACHE["nc"] = build_program()
    nc = _CACHE["nc"]

    in_maps = prep_inputs(feat, gallery_features)
    res = run_bass_kernel_spmd(nc, in_maps, list(range(M)))
    cvs = [res.results[c]["candv"] for c in range(M)]
    cis = [res.results[c]["candi"] for c in range(M)]
    return merge_outputs(cvs, cis, k, feat, gallery_features)
